# revision 21
# baseline (speedup 1.0000x reference)
"""Distributed ImprovedDilatedAttention on 8 Trainium2 NeuronCores.

Problem: [2, 4096, 12, 64] q/k/v, 3 head groups with (segment, dilation) in
[(1024,1), (2048,2), (4096,4)]. Each (group, batch, segment, head) pair is an
independent dense 1024x1024 attention over head_dim 64 (m = g/r = 1024 for
every group): 56 problems total, 7 per core.

Host side packs one bf16 input block per problem, [128, 2568] = qT | kT | vp:
  qT [128, 1024] = Q^T duplicated into both partition halves (row tiling)
  kT [128, 1024] = K^T duplicated likewise (stationary operand for S^T)
  vp [128, 8, 65] = V' chunks, V' = [V | 1]; vp[j, c, :] = V'[c*128 + j]

Device: RAW BASS (no Tile framework) three-engine softmax pipeline with ~9
hand-managed counting semaphores.  Rationale vs the earlier Tile version:
Tile's vector-clock sem file forced a ~10us end-of-NEFF reset loop (256
individual sem zeroing instructions) plus per-instruction sem traffic; raw
bass replaces that with one gpsimd RANGE_CLEAR.  Manual instruction order
also lets us (a) warm the PE HAM clock gate with dummy matmuls during the
input-DMA window (cold K=4/8 costs 2x on every matmul for the first ~10us
otherwise) and (b) start the first real matmul as soon as the first split
DMA piece lands.

Per problem (identical math to the Tile version):
  S^T[kj, qi] = sum_d K^T[d,kj] Q^T[d,qi]  in 16 (kj block x qi half) units,
      PE row strips 0-63/64-127 alternating so pairs stream concurrently;
      grouped into 8 chunks of [128,1024] fp32 rotating over 3 PSUM slots.
  E = exp(S/8): even chunks on ScalarE (exact ACTIVATE Exp), odd chunks on
      VectorE (Schraudolph tensor_scalar fp32->int16 whose bits ARE
      bf16(2^z); the (kj block, qi half)->engine map keeps every query at
      exactly 4 approximated key blocks).
  out[m, qi] = sum_kj V'[kj, m] E[kj, qi]: V' stationary (8 65-col
      LDWEIGHTS), E moving at N=512, accumulated into 2 single-bank
      [65,512] PSUM tiles (one per qi half), quads interleaved with the S
      chunks two groups back (baseline's pend schedule).  ScalarE copies
      half 0, VectorE half 1 to SBUF; one [65,1024] DMA out per problem.
out rows 0:64 are the unnormalized O^T, row 64 is sumexp.  Host divides and
scatters into the dilated positions (zeros elsewhere).
"""

import numpy as np

B, N, H, D = 2, 4096, 12, 64
SEG = [1024, 2048, 4096]
DIL = [1, 2, 4]
NGROUPS = 3
HPG = H // NGROUPS  # 4 heads per group
M = 1024            # dilated tokens per segment (g // r, same for all groups)
NPROB = 56
NCORES = 8
PPC = NPROB // NCORES  # 7 problems per core

_CACHE = {}

# Schraudolph exp on VectorE: bits_i16 = trunc(S * EXP_A + EXP_B); the int16
# bit pattern equals bf16(exp(S/8)) under a piecewise-linear 2^f approx.
# EXP_A = 16*log2(e) (the /8 score scale folded in); EXP_B tuned numerically
# (127<<7 minus ~5.1 PWL-centering correction, assuming truncating convert).
EXP_A = float(np.float32(16.0 * np.log2(np.e)))
EXP_B = 16250.875

# unit = (kj block j, qi half qh); SEQ is the emission order. Chunks are the
# 8 consecutive pairs; even chunks -> ScalarE exact exp, odd chunks ->
# VectorE Schraudolph. DVE cells are {(odd j, 0)} + {(even j, 1)}: each qi
# half sees exactly 4 approximated kj blocks. Units inside a chunk alternate
# row strips (j parity) so their matmuls overlap on the PE.
SEQ = [(0, 0), (2, 0),   # c0 ACT
       (1, 0), (3, 0),   # c1 DVE
       (4, 0), (6, 0),   # c2 ACT
       (5, 0), (7, 0),   # c3 DVE
       (1, 1), (3, 1),   # c4 ACT
       (0, 1), (2, 1),   # c5 DVE
       (5, 1), (7, 1),   # c6 ACT
       (4, 1), (6, 1)]   # c7 DVE

N_WARM = 12   # HAM warmup matmuls (N=256 each) before the first real S MM
# p0 pipeline-fill fillers: keep the PE busy through the first problem's
# exp-wait stalls so the HAM clock gate doesn't re-throttle mid-ramp.
FILL = {1: 4, 2: 3, 3: 2}   # g -> filler MMs before that group (p0 only)


def _bf16():
    import ml_dtypes

    return ml_dtypes.bfloat16


def _groups():
    for i, (g, r) in enumerate(zip(SEG, DIL)):
        yield i, g, r, i % r, N // g


def _pack(query, key, value):
    """-> per-problem input blocks [56, 128, 2568] = qT | kT | vp (bf16)."""
    bf16 = _bf16()
    qs, ks, vs = [], [], []
    for i, g, r, off, s in _groups():
        idx = off + r * np.arange(g // r)
        hsl = slice(i * HPG, (i + 1) * HPG)

        def grab(x):
            return x.reshape(B, s, g, H, D)[:, :, idx][:, :, :, hsl, :]

        qg = grab(query)  # [B, s, m, hpg, D]
        kg = grab(key)
        vg = grab(value)
        qT = np.ascontiguousarray(qg.transpose(0, 1, 3, 4, 2)).reshape(-1, D, M)
        kT = np.ascontiguousarray(kg.transpose(0, 1, 3, 4, 2)).reshape(-1, D, M)
        # duplicate into both partition halves for 2-way PE row tiling
        qs.append(np.concatenate([qT, qT], axis=1))  # [n, 128, M]
        ks.append(np.concatenate([kT, kT], axis=1))
        v65 = np.concatenate(
            [vg, np.ones((*vg.shape[:-1], 1), np.float32)], axis=-1
        )  # [B, s, m, hpg, 65]
        vp = np.ascontiguousarray(v65.transpose(0, 1, 3, 2, 4)).reshape(-1, M, 65)
        vp = np.ascontiguousarray(vp.reshape(-1, 8, 128, 65).transpose(0, 2, 1, 3))
        vs.append(vp)
    qTp = np.concatenate(qs).astype(bf16)   # [56, 128, 1024]
    kTp = np.concatenate(ks).astype(bf16)   # [56, 128, 1024]
    vpp = np.concatenate(vs).astype(bf16)   # [56, 128, 8, 65]
    return np.concatenate(
        [qTp, kTp, vpp.reshape(NPROB, 128, 520)], axis=2
    )  # [56, 128, 2568]


def _unpack(outT):
    """outT [56, 65, 1024] (m-row, qi-col) -> full output."""
    o = outT.transpose(0, 2, 1)  # [56, qi, 65]
    o = o[:, :, :64] / o[:, :, 64:65]  # [56, qi, 64]
    out = np.zeros((B, N, H, D), np.float32)
    ofs = 0
    for i, g, r, off, s in _groups():
        idx = off + r * np.arange(g // r)
        n_i = B * s * HPG
        og = o[ofs : ofs + n_i].reshape(B, s, HPG, M, D).transpose(0, 1, 3, 2, 4)
        out.reshape(B, s, g, H, D)[:, :, idx, i * HPG : (i + 1) * HPG, :] = og
        ofs += n_i
    return out


def _quad_schedule():
    """Replicates the pend deque: returns flush order of (p, g) quads and,
    per (p, g), its 1-based global quad index."""
    from collections import deque

    pend = deque()
    order = []
    for p in range(PPC):
        for g in range(4):
            pend.append((p, g))
            while len(pend) > (2 if p < PPC - 1 else 1):
                order.append(pend.popleft())
    while pend:
        order.append(pend.popleft())
    qidx = {pg: i + 1 for i, pg in enumerate(order)}
    return order, qidx


def _build(for_hw=True):
    import concourse.bacc as bacc
    import concourse.bass as bass
    import concourse.mybir as mybir
    from contextlib import ExitStack

    f32 = mybir.dt.float32
    i16 = mybir.dt.int16
    bf = mybir.dt.bfloat16
    Exp = mybir.ActivationFunctionType.Exp

    nc = bacc.Bacc("TRN2", target_bir_lowering=False, debug=False,
                   enable_asserts=False)
    inx = nc.dram_tensor("inx", [PPC, 128, 2568], bf, kind="ExternalInput").ap()
    outT = nc.dram_tensor("outT", [PPC, 65, 1024], f32, kind="ExternalOutput").ap()

    ctx = ExitStack()
    sb = lambda name, shape, dt: ctx.enter_context(nc.sbuf_tensor(name, shape, dt))
    psm = lambda name, shape, dt: ctx.enter_context(nc.psum_tensor(name, shape, dt))

    its = [sb(f"it{i}", [128, 2568], bf) for i in range(PPC)]
    eSb = [sb(f"eS{i}", [128, 8192], bf) for i in range(2)]
    otb = [sb(f"ot{i}", [65, 1024], f32) for i in range(2)]
    wz = sb("wz", [128, 576], bf)   # zeroed scratch: warmup stationary/moving
    zb = sb("zb", [128, 1], f32)    # zero bias for ACTIVATE Exp

    spool = [psm(f"sch{i}", [128, 1024], f32) for i in range(3)]  # S chunks
    pv = [psm(f"pv{i}", [65, 512], f32) for i in range(2)]        # PV accum

    sem_names = (["sA0", "s0b", "s0c", "s0d", "s0e"]
                 + [f"sP{p}" for p in range(1, PPC)]
                 + ["sW", "sS", "sA", "sV", "sPV", "sC0", "sC1",
                    "sOUTe", "sOUTo", "sT6", "sT7"])
    sems = {name: nc.alloc_semaphore(name) for name in sem_names}
    sA0, s0b, s0c, s0d, s0e = (sems[k] for k in
                               ("sA0", "s0b", "s0c", "s0d", "s0e"))
    sP = {p: sems[f"sP{p}"] for p in range(1, PPC)}
    sW, sS, sA, sV, sPV, sC0, sC1 = (
        sems[k] for k in ("sW", "sS", "sA", "sV", "sPV", "sC0", "sC1"))
    sT6, sT7 = sems["sT6"], sems["sT7"]
    sOUT = [sems["sOUTe"], sems["sOUTo"]]

    quad_order, qidx = _quad_schedule()

    # exp index formulas: 1-based completion counts on each exp engine
    def act_cnt(p, c):  # c even
        return 4 * p + c // 2 + 1

    def dve_cnt(p, c):  # c odd
        return 4 * p + (c - 1) // 2 + 1

    # which quads have been flushed strictly before tensor-program point
    # (p, g) finishes its S MMs -> used for nothing; waits use qidx directly.

    def build_tensor(te):
        # An instruction carries at most one wait condition; extras become
        # standalone EVENT_SEMAPHORE waits emitted just before it.
        def spill(waits):
            for s, v in waits[1:]:
                te.wait_ge(s, v)
            return waits[0] if waits else None

        # --- HAM warmup: junk matmuls into pv[0] while input DMA flies.
        first = te.matmul(pv[0][:, 0:256], wz[:, 0:65], wz[:, 64:320],
                          start=True, stop=True)
        first._wait_ge(sW, 1)
        for _ in range(N_WARM - 1):
            te.matmul(pv[0][:, 0:256], wz[:, 0:65], wz[:, 64:320],
                      start=True, stop=True)

        def emit_quad(p, g):
            # PV quad for chunk pair (2g, 2g+1) of problem p
            eS = eSb[p % 2]
            vpt = its[p][:, 2048:2568].rearrange("a (c m) -> a c m", m=65)
            last = None
            for c in (2 * g, 2 * g + 1):
                for k, u in enumerate((2 * c, 2 * c + 1)):
                    c8, qh = SEQ[u]
                    waits = []
                    if p == PPC - 1 and g == 3:
                        # final drain: exps of chunks 6/7 are split in half;
                        # each cell waits only on its own half
                        if k == 0:
                            waits.append((sT6, 1) if c % 2 == 0 else (sT7, 1))
                        else:
                            if c % 2 == 0:
                                waits.append((sA, act_cnt(p, c)))
                            else:
                                waits.append((sV, dve_cnt(p, c)))
                    elif k == 0:  # exp of chunk c must be done
                        if c % 2 == 0:
                            waits.append((sA, act_cnt(p, c)))
                        else:
                            waits.append((sV, dve_cnt(p, c)))
                    if u == 0 and p >= 1:
                        waits.append((sC0, p))   # pv0 copy of p-1 done
                    if u == 8 and p >= 1:
                        waits.append((sC1, p))   # pv1 copy of p-1 done
                    if u == 0 and p == 0:
                        waits.append((s0e, 16))  # vp piece of p0 landed
                    w0 = spill(waits)
                    mm = te.matmul(
                        pv[qh][:, :],
                        vpt[:, c8, :],
                        eS[:, u * 512:(u + 1) * 512],
                        start=(u % 8 == 0), stop=(u % 8 == 7),
                    )
                    if w0:
                        mm._wait_ge(*w0)
                    last = mm
            last.then_inc(sPV, 1)

        from collections import deque

        pend = deque()
        for p in range(PPC):
            it = its[p]
            qt = it[:, 0:1024]
            kt = it[:, 1024:2048]
            for g in range(4):
                if p == 0 and g in FILL:
                    for _ in range(FILL[g]):
                        te.matmul(pv[1][:, 0:256], wz[:, 0:65], wz[:, 64:320],
                                  start=True, stop=True)
                chunks = (2 * g, 2 * g + 1)
                for u2 in range(2):
                    for ci, c in enumerate(chunks):
                        j, qh = SEQ[2 * c + u2]
                        st64 = (j % 2) * 64
                        G = 8 * p + c  # global chunk index
                        sch = spool[G % 3]
                        waits = []
                        if u2 == 0 and G >= 3:
                            # PSUM slot reuse: exp of chunk G-3 done
                            pp, cc = divmod(G - 3, 8)
                            if cc % 2 == 0:
                                waits.append((sA, act_cnt(pp, cc)))
                            else:
                                waits.append((sV, dve_cnt(pp, cc)))
                        # input gating (first MM of relevant region only)
                        if p == 0 and u2 == 0 and ci == 0:
                            if g == 0:
                                waits.append((sA0, 16))  # qt half 0
                                waits.append((s0b, 16))  # kt blocks 0-3
                            elif g == 1:
                                waits.append((s0c, 16))  # kt blocks 4-7
                            elif g == 2:
                                waits.append((s0d, 16))  # qt half 1
                        if p >= 1 and g == 0 and u2 == 0 and ci == 0:
                            waits.append((sP[p], 16))
                        w0 = spill(waits)
                        mm = te.matmul(
                            sch[:, u2 * 512:(u2 + 1) * 512],
                            kt[st64:st64 + 64, j * 128:(j + 1) * 128],
                            qt[st64:st64 + 64, qh * 512:(qh + 1) * 512],
                            start=True, stop=True,
                            tile_position=(st64, 0),
                        )
                        if w0:
                            mm._wait_ge(*w0)
                        if u2 == 1:
                            mm.then_inc(sS, 1)  # chunk done (pc-monotone)
                pend.append((p, g))
                while len(pend) > (2 if p < PPC - 1 else 1):
                    emit_quad(*pend.popleft())
        while pend:
            emit_quad(*pend.popleft())

    def build_scalar(sc):
        # p0 early piece: qt cols 0:512 on the Act HWDGE ring
        sc.dma_start(its[0][:, 0:512], inx[0][:, 0:512]).then_inc(sA0, 16)
        for p in range(PPC):
            eS = eSb[p % 2]
            for c in (0, 2, 4, 6):
                if p == 0 and c == 0:
                    sc.wait_ge(sW, 2)          # zero-bias memset done
                if c == 0 and p >= 2:
                    sc.wait_ge(sPV, qidx[(p - 2, 3)])  # eS buffer free
                if p == PPC - 1 and c == 6:
                    # final chunk: halves, so the last PV quad starts early
                    a = sc.activation(
                        eS[:, 2 * c * 512:(2 * c + 1) * 512],
                        spool[(8 * p + c) % 3][:, 0:512],
                        Exp, bias=zb[:, 0:1], scale=0.125,
                    )
                    a._wait_ge(sS, 8 * p + c + 1)
                    a.then_inc(sT6, 1)
                    a = sc.activation(
                        eS[:, (2 * c + 1) * 512:(2 * c + 2) * 512],
                        spool[(8 * p + c) % 3][:, 512:1024],
                        Exp, bias=zb[:, 0:1], scale=0.125,
                    )
                    a.then_inc(sA, 1)
                    continue
                a = sc.activation(
                    eS[:, 2 * c * 512:(2 * c + 2) * 512],
                    spool[(8 * p + c) % 3][:, :],
                    Exp, bias=zb[:, 0:1], scale=0.125,
                )
                a._wait_ge(sS, 8 * p + c + 1)
                a.then_inc(sA, 1)
            if p >= 2:
                sc.wait_ge(sOUT[p % 2], 16 * (p // 2))  # ot buffer free
            cp = sc.copy(otb[p % 2][:, 0:512], pv[0][:, :])
            cp._wait_ge(sPV, qidx[(p, 1)])
            cp.then_inc(sC0, 1)

    def build_vector(ve):
        for p in range(PPC):
            eS = eSb[p % 2]
            for c in (1, 3, 5, 7):
                if c == 1 and p >= 2:
                    ve.wait_ge(sPV, qidx[(p - 2, 3)])  # eS buffer free
                if p == PPC - 1 and c == 7:
                    t = ve.tensor_scalar(
                        out=eS[:, 2 * c * 512:(2 * c + 1) * 512].bitcast(i16),
                        in0=spool[(8 * p + c) % 3][:, 0:512],
                        scalar1=EXP_A, scalar2=EXP_B,
                        op0=mybir.AluOpType.mult, op1=mybir.AluOpType.add,
                    )
                    t._wait_ge(sS, 8 * p + c + 1)
                    t.then_inc(sT7, 1)
                    t = ve.tensor_scalar(
                        out=eS[:, (2 * c + 1) * 512:(2 * c + 2) * 512]
                        .bitcast(i16),
                        in0=spool[(8 * p + c) % 3][:, 512:1024],
                        scalar1=EXP_A, scalar2=EXP_B,
                        op0=mybir.AluOpType.mult, op1=mybir.AluOpType.add,
                    )
                    t.then_inc(sV, 1)
                    continue
                t = ve.tensor_scalar(
                    out=eS[:, 2 * c * 512:(2 * c + 2) * 512].bitcast(i16),
                    in0=spool[(8 * p + c) % 3][:, :],
                    scalar1=EXP_A, scalar2=EXP_B,
                    op0=mybir.AluOpType.mult, op1=mybir.AluOpType.add,
                )
                t._wait_ge(sS, 8 * p + c + 1)
                t.then_inc(sV, 1)
                if c == 5 and p >= 1:
                    # pv1 copy of p-1: placed late (after exp(p,c5), not at
                    # the top of problem p) so its wait on quad (p-1,g3) —
                    # which runs ~now on the PE — can't stall the strict
                    # FIFO queue ahead of independent exps.
                    q = p - 1
                    if q >= 2:
                        ve.wait_ge(sOUT[q % 2], 16 * ((q - 2) // 2 + 1))
                    cp = ve.tensor_copy(out=otb[q % 2][:, 512:1024],
                                        in_=pv[1][:, :])
                    cp._wait_ge(sPV, qidx[(q, 3)])
                    cp.then_inc(sC1, 1)
        q = PPC - 1
        ve.wait_ge(sOUT[q % 2], 16 * ((q - 2) // 2 + 1))
        cp = ve.tensor_copy(out=otb[q % 2][:, 512:1024], in_=pv[1][:, :])
        cp._wait_ge(sPV, qidx[(q, 3)])
        cp.then_inc(sC1, 1)

    def build_sync(sy):
        # p0 pieces in need-order: kt j0-3, kt j4-7, qt half1, vp
        sy.dma_start(its[0][:, 1024:1536], inx[0][:, 1024:1536]).then_inc(s0b, 16)
        sy.dma_start(its[0][:, 1536:2048], inx[0][:, 1536:2048]).then_inc(s0c, 16)
        sy.dma_start(its[0][:, 512:1024], inx[0][:, 512:1024]).then_inc(s0d, 16)
        sy.dma_start(its[0][:, 2048:2568], inx[0][:, 2048:2568]).then_inc(s0e, 16)
        for p in range(1, PPC):
            sy.dma_start(its[p][:, :], inx[p][:, :]).then_inc(sP[p], 16)
        for p in range(PPC):
            if p == PPC - 1:
                d = sy.dma_start(outT[p][:, 0:512], otb[p % 2][:, 0:512])
                d._wait_ge(sC0, p + 1)
                d.then_inc(sOUT[p % 2], 16)
                d = sy.dma_start(outT[p][:, 512:1024],
                                 otb[p % 2][:, 512:1024])
                d._wait_ge(sC1, p + 1)
                d.then_inc(sOUT[p % 2], 16)
            else:
                sy.wait_ge(sC1, p + 1)
                d = sy.dma_start(outT[p][:, :], otb[p % 2][:, :])
                d._wait_ge(sC0, p + 1)
                d.then_inc(sOUT[p % 2], 16)

    def build_gpsimd(g):
        g.memset(wz[:, :], 0.0).then_inc(sW, 1)
        g.memset(zb[:, :], 0.0).then_inc(sW, 1)

    with nc.Block(name="dila") as blk:
        blk.gpsimd(build_gpsimd)
        blk.sync(build_sync)
        blk.scalar(build_scalar)
        blk.vector(build_vector)
        blk.tensor(build_tensor)

    # Block exit emitted an all-engine barrier; restore every kernel sem to
    # zero for NEFF re-execution with one gpsimd range-clear (the thing that
    # cost the Tile version ~10us of per-sem resets).
    nc.clear_and_free_semaphores(list(sems.values()))

    ctx.close()
    nc.compile()
    if for_hw:
        from concourse.bass_interp import get_hw_module

        nc.m = get_hw_module(nc.m)
    return nc


def _numpy_fallback(query, key, value, causal):
    out = np.zeros((B, N, H, D), np.float32)
    for i, g, r, off, s in _groups():
        idx = off + r * np.arange(g // r)
        hsl = slice(i * HPG, (i + 1) * HPG)
        q = query.reshape(B, s, g, H, D)[:, :, idx][:, :, :, hsl, :]
        k = key.reshape(B, s, g, H, D)[:, :, idx][:, :, :, hsl, :]
        v = value.reshape(B, s, g, H, D)[:, :, idx][:, :, :, hsl, :]
        scores = np.einsum("bsqhd,bskhd->bshqk", q, k) / np.sqrt(D).astype(np.float32)
        if causal:
            mask = np.tril(np.ones((g // r, g // r), dtype=bool))
            scores = np.where(mask, scores, np.float32(np.finfo(np.float32).min))
        scores -= scores.max(axis=-1, keepdims=True)
        p = np.exp(scores)
        p /= p.sum(axis=-1, keepdims=True)
        o = np.einsum("bshqk,bskhd->bsqhd", p, v)
        out.reshape(B, s, g, H, D)[:, :, idx, hsl, :] = o
    return out


def _in_maps(query, key, value):
    inx = _pack(query, key, value)
    return [
        {"inx": np.ascontiguousarray(inx[k * PPC : (k + 1) * PPC])}
        for k in range(NCORES)
    ]


def kernel(query, key, value, is_causal):
    query = np.asarray(query, dtype=np.float32)
    key = np.asarray(key, dtype=np.float32)
    value = np.asarray(value, dtype=np.float32)
    causal = bool(np.asarray(is_causal).item()) if np.ndim(is_causal) == 0 else bool(
        is_causal
    )
    if causal:
        return _numpy_fallback(query, key, value, causal)

    from concourse import bass_utils

    if "nc" not in _CACHE:
        _CACHE["nc"] = _build()
    nc = _CACHE["nc"]

    res = bass_utils.run_bass_kernel_spmd(
        nc, _in_maps(query, key, value), core_ids=list(range(NCORES))
    )
    outT = np.concatenate([res.results[k]["outT"] for k in range(NCORES)])
    return _unpack(outT)


# revision 22
# speedup vs baseline: 1.0039x; 1.0039x over previous
"""Distributed ImprovedDilatedAttention on 8 Trainium2 NeuronCores.

Problem: [2, 4096, 12, 64] q/k/v, 3 head groups with (segment, dilation) in
[(1024,1), (2048,2), (4096,4)]. Each (group, batch, segment, head) pair is an
independent dense 1024x1024 attention over head_dim 64 (m = g/r = 1024 for
every group): 56 problems total, 7 per core.

Host side packs one bf16 input block per problem, [128, 2568] = qT | kT | vp:
  qT [128, 1024] = Q^T duplicated into both partition halves (row tiling)
  kT [128, 1024] = K^T duplicated likewise (stationary operand for S^T)
  vp [128, 8, 65] = V' chunks, V' = [V | 1]; vp[j, c, :] = V'[c*128 + j]

Device: RAW BASS (no Tile framework) three-engine softmax pipeline with ~9
hand-managed counting semaphores.  Rationale vs the earlier Tile version:
Tile's vector-clock sem file forced a ~10us end-of-NEFF reset loop (256
individual sem zeroing instructions) plus per-instruction sem traffic; raw
bass replaces that with one gpsimd RANGE_CLEAR.  Manual instruction order
also lets us (a) warm the PE HAM clock gate with dummy matmuls during the
input-DMA window (cold K=4/8 costs 2x on every matmul for the first ~10us
otherwise) and (b) start the first real matmul as soon as the first split
DMA piece lands.

Per problem (identical math to the Tile version):
  S^T[kj, qi] = sum_d K^T[d,kj] Q^T[d,qi]  in 16 (kj block x qi half) units,
      PE row strips 0-63/64-127 alternating so pairs stream concurrently;
      grouped into 8 chunks of [128,1024] fp32 rotating over 3 PSUM slots.
  E = exp(S/8): even chunks on ScalarE (exact ACTIVATE Exp), odd chunks on
      VectorE (Schraudolph tensor_scalar fp32->int16 whose bits ARE
      bf16(2^z); the (kj block, qi half)->engine map keeps every query at
      exactly 4 approximated key blocks).
  out[m, qi] = sum_kj V'[kj, m] E[kj, qi]: V' stationary (8 65-col
      LDWEIGHTS), E moving at N=512, accumulated into 2 single-bank
      [65,512] PSUM tiles (one per qi half), quads interleaved with the S
      chunks two groups back (baseline's pend schedule).  ScalarE copies
      half 0, VectorE half 1 to SBUF; one [65,1024] DMA out per problem.
out rows 0:64 are the unnormalized O^T, row 64 is sumexp.  Host divides and
scatters into the dilated positions (zeros elsewhere).
"""

import numpy as np

B, N, H, D = 2, 4096, 12, 64
SEG = [1024, 2048, 4096]
DIL = [1, 2, 4]
NGROUPS = 3
HPG = H // NGROUPS  # 4 heads per group
M = 1024            # dilated tokens per segment (g // r, same for all groups)
NPROB = 56
NCORES = 8
PPC = NPROB // NCORES  # 7 problems per core

_CACHE = {}

# Schraudolph exp on VectorE: bits_i16 = trunc(S * EXP_A + EXP_B); the int16
# bit pattern equals bf16(exp(S/8)) under a piecewise-linear 2^f approx.
# EXP_A = 16*log2(e) (the /8 score scale folded in); EXP_B tuned numerically
# (127<<7 minus ~5.1 PWL-centering correction, assuming truncating convert).
EXP_A = float(np.float32(16.0 * np.log2(np.e)))
EXP_B = 16250.875

# unit = (kj block j, qi half qh); SEQ is the emission order. Chunks are the
# 8 consecutive pairs; even chunks -> ScalarE exact exp, odd chunks ->
# VectorE Schraudolph. DVE cells are {(odd j, 0)} + {(even j, 1)}: each qi
# half sees exactly 4 approximated kj blocks. Units inside a chunk alternate
# row strips (j parity) so their matmuls overlap on the PE.
SEQ = [(0, 0), (2, 0),   # c0 ACT
       (1, 0), (3, 0),   # c1 DVE
       (4, 0), (6, 0),   # c2 ACT
       (5, 0), (7, 0),   # c3 DVE
       (1, 1), (3, 1),   # c4 ACT
       (0, 1), (2, 1),   # c5 DVE
       (5, 1), (7, 1),   # c6 ACT
       (4, 1), (6, 1)]   # c7 DVE

N_WARM = 12   # HAM warmup matmuls (N=256 each) before the first real S MM
# p0 pipeline-fill fillers: keep the PE busy through the first problem's
# exp-wait stalls so the HAM clock gate doesn't re-throttle mid-ramp.
FILL = {1: 4, 2: 3, 3: 2}   # g -> filler MMs before that group (p0 only)


def _bf16():
    import ml_dtypes

    return ml_dtypes.bfloat16


def _groups():
    for i, (g, r) in enumerate(zip(SEG, DIL)):
        yield i, g, r, i % r, N // g


def _pack(query, key, value):
    """-> per-problem input blocks [56, 128, 2568] = qT | kT | vp (bf16)."""
    bf16 = _bf16()
    qs, ks, vs = [], [], []
    for i, g, r, off, s in _groups():
        idx = off + r * np.arange(g // r)
        hsl = slice(i * HPG, (i + 1) * HPG)

        def grab(x):
            return x.reshape(B, s, g, H, D)[:, :, idx][:, :, :, hsl, :]

        qg = grab(query)  # [B, s, m, hpg, D]
        kg = grab(key)
        vg = grab(value)
        qT = np.ascontiguousarray(qg.transpose(0, 1, 3, 4, 2)).reshape(-1, D, M)
        kT = np.ascontiguousarray(kg.transpose(0, 1, 3, 4, 2)).reshape(-1, D, M)
        # duplicate into both partition halves for 2-way PE row tiling
        qs.append(np.concatenate([qT, qT], axis=1))  # [n, 128, M]
        ks.append(np.concatenate([kT, kT], axis=1))
        v65 = np.concatenate(
            [vg, np.ones((*vg.shape[:-1], 1), np.float32)], axis=-1
        )  # [B, s, m, hpg, 65]
        vp = np.ascontiguousarray(v65.transpose(0, 1, 3, 2, 4)).reshape(-1, M, 65)
        vp = np.ascontiguousarray(vp.reshape(-1, 8, 128, 65).transpose(0, 2, 1, 3))
        vs.append(vp)
    qTp = np.concatenate(qs).astype(bf16)   # [56, 128, 1024]
    kTp = np.concatenate(ks).astype(bf16)   # [56, 128, 1024]
    vpp = np.concatenate(vs).astype(bf16)   # [56, 128, 8, 65]
    return np.concatenate(
        [qTp, kTp, vpp.reshape(NPROB, 128, 520)], axis=2
    )  # [56, 128, 2568]


def _unpack(outT):
    """outT [56, 65, 1024] (m-row, qi-col) -> full output."""
    o = outT.transpose(0, 2, 1)  # [56, qi, 65]
    o = o[:, :, :64] / o[:, :, 64:65]  # [56, qi, 64]
    out = np.zeros((B, N, H, D), np.float32)
    ofs = 0
    for i, g, r, off, s in _groups():
        idx = off + r * np.arange(g // r)
        n_i = B * s * HPG
        og = o[ofs : ofs + n_i].reshape(B, s, HPG, M, D).transpose(0, 1, 3, 2, 4)
        out.reshape(B, s, g, H, D)[:, :, idx, i * HPG : (i + 1) * HPG, :] = og
        ofs += n_i
    return out


def _quad_schedule():
    """Replicates the pend deque: returns flush order of (p, g) quads and,
    per (p, g), its 1-based global quad index."""
    from collections import deque

    pend = deque()
    order = []
    for p in range(PPC):
        for g in range(4):
            pend.append((p, g))
            while len(pend) > (2 if p < PPC - 1 else 1):
                order.append(pend.popleft())
    while pend:
        order.append(pend.popleft())
    qidx = {pg: i + 1 for i, pg in enumerate(order)}
    return order, qidx


def _build(for_hw=True):
    import concourse.bacc as bacc
    import concourse.bass as bass
    import concourse.mybir as mybir
    from contextlib import ExitStack

    f32 = mybir.dt.float32
    i16 = mybir.dt.int16
    bf = mybir.dt.bfloat16
    Exp = mybir.ActivationFunctionType.Exp

    nc = bacc.Bacc("TRN2", target_bir_lowering=False, debug=False,
                   enable_asserts=False)
    inx = nc.dram_tensor("inx", [PPC, 128, 2568], bf, kind="ExternalInput").ap()
    outT = nc.dram_tensor("outT", [PPC, 65, 1024], f32, kind="ExternalOutput").ap()

    ctx = ExitStack()
    sb = lambda name, shape, dt: ctx.enter_context(nc.sbuf_tensor(name, shape, dt))
    psm = lambda name, shape, dt: ctx.enter_context(nc.psum_tensor(name, shape, dt))

    its = [sb(f"it{i}", [128, 2568], bf) for i in range(PPC)]
    eSb = [sb(f"eS{i}", [128, 8192], bf) for i in range(2)]
    otb = [sb(f"ot{i}", [65, 1024], f32) for i in range(2)]
    wz = sb("wz", [128, 576], bf)   # zeroed scratch: warmup stationary/moving
    zb = sb("zb", [128, 1], f32)    # zero bias for ACTIVATE Exp

    spool = [psm(f"sch{i}", [128, 1024], f32) for i in range(3)]  # S chunks
    pv = [psm(f"pv{i}", [65, 512], f32) for i in range(2)]        # PV accum

    sem_names = (["sA0", "s0b", "s0c", "s0d", "s0e"]
                 + [f"sP{p}" for p in range(1, PPC)]
                 + ["sW", "sS", "sA", "sV", "sPV", "sC0", "sC1",
                    "sOUTe", "sOUTo"])
    sems = {name: nc.alloc_semaphore(name) for name in sem_names}
    sA0, s0b, s0c, s0d, s0e = (sems[k] for k in
                               ("sA0", "s0b", "s0c", "s0d", "s0e"))
    sP = {p: sems[f"sP{p}"] for p in range(1, PPC)}
    sW, sS, sA, sV, sPV, sC0, sC1 = (
        sems[k] for k in ("sW", "sS", "sA", "sV", "sPV", "sC0", "sC1"))
    sOUT = [sems["sOUTe"], sems["sOUTo"]]

    quad_order, qidx = _quad_schedule()

    # exp index formulas: 1-based completion counts on each exp engine
    def act_cnt(p, c):  # c even
        return 4 * p + c // 2 + 1

    def dve_cnt(p, c):  # c odd
        return 4 * p + (c - 1) // 2 + 1

    # which quads have been flushed strictly before tensor-program point
    # (p, g) finishes its S MMs -> used for nothing; waits use qidx directly.

    def build_tensor(te):
        # An instruction carries at most one wait condition; extras become
        # standalone EVENT_SEMAPHORE waits emitted just before it.
        def spill(waits):
            for s, v in waits[1:]:
                te.wait_ge(s, v)
            return waits[0] if waits else None

        # --- HAM warmup: junk matmuls into pv[0] while input DMA flies.
        first = te.matmul(pv[0][:, 0:256], wz[:, 0:65], wz[:, 64:320],
                          start=True, stop=True)
        first._wait_ge(sW, 1)
        for _ in range(N_WARM - 1):
            te.matmul(pv[0][:, 0:256], wz[:, 0:65], wz[:, 64:320],
                      start=True, stop=True)

        def emit_quad(p, g):
            # PV quad for chunk pair (2g, 2g+1) of problem p
            eS = eSb[p % 2]
            vpt = its[p][:, 2048:2568].rearrange("a (c m) -> a c m", m=65)
            last = None
            for c in (2 * g, 2 * g + 1):
                for k, u in enumerate((2 * c, 2 * c + 1)):
                    c8, qh = SEQ[u]
                    waits = []
                    if k == 0:  # exp of chunk c must be done
                        if c % 2 == 0:
                            waits.append((sA, act_cnt(p, c)))
                        else:
                            waits.append((sV, dve_cnt(p, c)))
                    if u == 0 and p >= 1:
                        waits.append((sC0, p))   # pv0 copy of p-1 done
                    if u == 8 and p >= 1:
                        waits.append((sC1, p))   # pv1 copy of p-1 done
                    if u == 0 and p == 0:
                        waits.append((s0e, 16))  # vp piece of p0 landed
                    w0 = spill(waits)
                    mm = te.matmul(
                        pv[qh][:, :],
                        vpt[:, c8, :],
                        eS[:, u * 512:(u + 1) * 512],
                        start=(u % 8 == 0), stop=(u % 8 == 7),
                    )
                    if w0:
                        mm._wait_ge(*w0)
                    last = mm
            last.then_inc(sPV, 1)

        from collections import deque

        pend = deque()
        for p in range(PPC):
            it = its[p]
            qt = it[:, 0:1024]
            kt = it[:, 1024:2048]
            for g in range(4):
                if p == 0 and g in FILL:
                    for _ in range(FILL[g]):
                        te.matmul(pv[1][:, 0:256], wz[:, 0:65], wz[:, 64:320],
                                  start=True, stop=True)
                chunks = (2 * g, 2 * g + 1)
                for u2 in range(2):
                    for ci, c in enumerate(chunks):
                        j, qh = SEQ[2 * c + u2]
                        st64 = (j % 2) * 64
                        G = 8 * p + c  # global chunk index
                        sch = spool[G % 3]
                        waits = []
                        if u2 == 0 and G >= 3:
                            # PSUM slot reuse: exp of chunk G-3 done
                            pp, cc = divmod(G - 3, 8)
                            if cc % 2 == 0:
                                waits.append((sA, act_cnt(pp, cc)))
                            else:
                                waits.append((sV, dve_cnt(pp, cc)))
                        # input gating (first MM of relevant region only)
                        if p == 0 and u2 == 0 and ci == 0:
                            if g == 0:
                                waits.append((sA0, 16))  # qt half 0
                                waits.append((s0b, 16))  # kt blocks 0-3
                            elif g == 1:
                                waits.append((s0c, 16))  # kt blocks 4-7
                            elif g == 2:
                                waits.append((s0d, 16))  # qt half 1
                        if p >= 1 and g == 0 and u2 == 0 and ci == 0:
                            waits.append((sP[p], 16))
                        w0 = spill(waits)
                        mm = te.matmul(
                            sch[:, u2 * 512:(u2 + 1) * 512],
                            kt[st64:st64 + 64, j * 128:(j + 1) * 128],
                            qt[st64:st64 + 64, qh * 512:(qh + 1) * 512],
                            start=True, stop=True,
                            tile_position=(st64, 0),
                        )
                        if w0:
                            mm._wait_ge(*w0)
                        if u2 == 1:
                            mm.then_inc(sS, 1)  # chunk done (pc-monotone)
                pend.append((p, g))
                while len(pend) > (2 if p < PPC - 1 else 1):
                    emit_quad(*pend.popleft())
        while pend:
            emit_quad(*pend.popleft())

    def build_scalar(sc):
        # p0 early piece: qt cols 0:512 on the Act HWDGE ring
        sc.dma_start(its[0][:, 0:512], inx[0][:, 0:512]).then_inc(sA0, 16)
        for p in range(PPC):
            eS = eSb[p % 2]
            for c in (0, 2, 4, 6):
                if p == 0 and c == 0:
                    sc.wait_ge(sW, 2)          # zero-bias memset done
                if c == 0 and p >= 2:
                    sc.wait_ge(sPV, qidx[(p - 2, 3)])  # eS buffer free
                a = sc.activation(
                    eS[:, 2 * c * 512:(2 * c + 2) * 512],
                    spool[(8 * p + c) % 3][:, :],
                    Exp, bias=zb[:, 0:1], scale=0.125,
                )
                a._wait_ge(sS, 8 * p + c + 1)
                a.then_inc(sA, 1)
            if p >= 2:
                sc.wait_ge(sOUT[p % 2], 16 * (p // 2))  # ot buffer free
            cp = sc.copy(otb[p % 2][:, 0:512], pv[0][:, :])
            cp._wait_ge(sPV, qidx[(p, 1)])
            cp.then_inc(sC0, 1)

    def build_vector(ve):
        for p in range(PPC):
            eS = eSb[p % 2]
            for c in (1, 3, 5, 7):
                if c == 1 and p >= 2:
                    ve.wait_ge(sPV, qidx[(p - 2, 3)])  # eS buffer free
                t = ve.tensor_scalar(
                    out=eS[:, 2 * c * 512:(2 * c + 2) * 512].bitcast(i16),
                    in0=spool[(8 * p + c) % 3][:, :],
                    scalar1=EXP_A, scalar2=EXP_B,
                    op0=mybir.AluOpType.mult, op1=mybir.AluOpType.add,
                )
                t._wait_ge(sS, 8 * p + c + 1)
                t.then_inc(sV, 1)
                if c == 5 and p >= 1:
                    # pv1 copy of p-1: placed late (after exp(p,c5), not at
                    # the top of problem p) so its wait on quad (p-1,g3) —
                    # which runs ~now on the PE — can't stall the strict
                    # FIFO queue ahead of independent exps.
                    q = p - 1
                    if q >= 2:
                        ve.wait_ge(sOUT[q % 2], 16 * ((q - 2) // 2 + 1))
                    cp = ve.tensor_copy(out=otb[q % 2][:, 512:1024],
                                        in_=pv[1][:, :])
                    cp._wait_ge(sPV, qidx[(q, 3)])
                    cp.then_inc(sC1, 1)
        q = PPC - 1
        ve.wait_ge(sOUT[q % 2], 16 * ((q - 2) // 2 + 1))
        cp = ve.tensor_copy(out=otb[q % 2][:, 512:1024], in_=pv[1][:, :])
        cp._wait_ge(sPV, qidx[(q, 3)])
        cp.then_inc(sC1, 1)

    def build_sync(sy):
        # p0 pieces in need-order: kt j0-3, kt j4-7, qt half1, vp
        sy.dma_start(its[0][:, 1024:1536], inx[0][:, 1024:1536]).then_inc(s0b, 16)
        sy.dma_start(its[0][:, 1536:2048], inx[0][:, 1536:2048]).then_inc(s0c, 16)
        sy.dma_start(its[0][:, 512:1024], inx[0][:, 512:1024]).then_inc(s0d, 16)
        sy.dma_start(its[0][:, 2048:2568], inx[0][:, 2048:2568]).then_inc(s0e, 16)
        for p in range(1, PPC):
            sy.dma_start(its[p][:, :], inx[p][:, :]).then_inc(sP[p], 16)
        for p in range(PPC):
            sy.wait_ge(sC1, p + 1)
            d = sy.dma_start(outT[p][:, :], otb[p % 2][:, :])
            d._wait_ge(sC0, p + 1)
            d.then_inc(sOUT[p % 2], 16)

    def build_gpsimd(g):
        g.memset(wz[:, :], 0.0).then_inc(sW, 1)
        g.memset(zb[:, :], 0.0).then_inc(sW, 1)

    with nc.Block(name="dila") as blk:
        blk.gpsimd(build_gpsimd)
        blk.sync(build_sync)
        blk.scalar(build_scalar)
        blk.vector(build_vector)
        blk.tensor(build_tensor)

    # Block exit emitted an all-engine barrier; restore every kernel sem to
    # zero for NEFF re-execution with one gpsimd range-clear (the thing that
    # cost the Tile version ~10us of per-sem resets).
    nc.clear_and_free_semaphores(list(sems.values()))

    ctx.close()
    nc.compile()
    if for_hw:
        from concourse.bass_interp import get_hw_module

        nc.m = get_hw_module(nc.m)
    return nc


def _numpy_fallback(query, key, value, causal):
    out = np.zeros((B, N, H, D), np.float32)
    for i, g, r, off, s in _groups():
        idx = off + r * np.arange(g // r)
        hsl = slice(i * HPG, (i + 1) * HPG)
        q = query.reshape(B, s, g, H, D)[:, :, idx][:, :, :, hsl, :]
        k = key.reshape(B, s, g, H, D)[:, :, idx][:, :, :, hsl, :]
        v = value.reshape(B, s, g, H, D)[:, :, idx][:, :, :, hsl, :]
        scores = np.einsum("bsqhd,bskhd->bshqk", q, k) / np.sqrt(D).astype(np.float32)
        if causal:
            mask = np.tril(np.ones((g // r, g // r), dtype=bool))
            scores = np.where(mask, scores, np.float32(np.finfo(np.float32).min))
        scores -= scores.max(axis=-1, keepdims=True)
        p = np.exp(scores)
        p /= p.sum(axis=-1, keepdims=True)
        o = np.einsum("bshqk,bskhd->bsqhd", p, v)
        out.reshape(B, s, g, H, D)[:, :, idx, hsl, :] = o
    return out


def _in_maps(query, key, value):
    inx = _pack(query, key, value)
    return [
        {"inx": np.ascontiguousarray(inx[k * PPC : (k + 1) * PPC])}
        for k in range(NCORES)
    ]


def kernel(query, key, value, is_causal):
    query = np.asarray(query, dtype=np.float32)
    key = np.asarray(key, dtype=np.float32)
    value = np.asarray(value, dtype=np.float32)
    causal = bool(np.asarray(is_causal).item()) if np.ndim(is_causal) == 0 else bool(
        is_causal
    )
    if causal:
        return _numpy_fallback(query, key, value, causal)

    from concourse import bass_utils

    if "nc" not in _CACHE:
        _CACHE["nc"] = _build()
    nc = _CACHE["nc"]

    res = bass_utils.run_bass_kernel_spmd(
        nc, _in_maps(query, key, value), core_ids=list(range(NCORES))
    )
    outT = np.concatenate([res.results[k]["outT"] for k in range(NCORES)])
    return _unpack(outT)
